# revision 20
# baseline (speedup 1.0000x reference)
# Distributed causal multi-head attention for 8 TRN2 NeuronCores.
#
# Problem: B=1, S=4096, D=768, H=12 heads, d_head=64, fp32 I/O.
#   out = softmax(causal((x_q Wq^T)(x_k Wk^T)^T / 8)) (x_v Wv^T) Wo^T  (+biases)
#
# Sharding: sequence-parallel over queries.  The 32 q-tiles (128 rows each)
# are dealt to cores with stride 8: core i owns q-tiles {i, 8+i, 16+i, 24+i}
# ("slots" j=0..3).  To keep the graph SPMD-identical across cores, slot j
# statically processes n_j = 8j+8 key-tiles; keys beyond a core's causal
# limit are zeroed in the exp domain with a per-core 0/1 mask (data, not
# structure).
#
# Attention runs PAIR-major: pair 0 = slots (0,1) over key tiles 0-15,
# pair 1 = slots (2,3) over key tiles 0-31.  For key tiles where both
# slots of a pair are causally active, one scores matmul streams both
# slots' q columns against a single khT stationary load, halving the
# LDWEIGHTS traffic that dominated the slot-major version.  Scores land
# transposed ([keys, q]); exp output feeds the PV matmul as the stationary
# operand (streaming V's 65 columns incl. a ones-column that yields the
# softmax denominator per q partition).  Work is pipelined at band (8
# key-tile) granularity: scores(band+1) overlaps exp/mask/PV(band), and
# pair-1 head-pairs (scalar-engine heavy) are interleaved with pair-0's
# remaining projection work (PE heavy) to keep both engines fed.
#
# K/V distribution: collectives have a fixed ~58us startup before the
# first transfer can complete, so only the latest-needed bands go through
# them: K tiles 16-31 and V tiles 16-31 are sharded (core c projects tiles
# {16+c},{24+c}) and exchanged via four AllGathers ordered K2, V2, K3, V3
# to match first use.  K/V tiles 0-15 are projected redundantly per core,
# interleaved into the pair-0 head loop (K by feature chunk, V by head
# half).

import numpy as np
import ml_dtypes

import concourse.bass as bass
import concourse.mybir as mybir
import concourse.tile as tile
from concourse import bacc
from concourse import bass_utils

P = 128
S = 4096
D = 768
H = 12
DH = 64
N_CORES = 8
NQ = 512                      # q rows per core
N_SLOTS = 4                   # q-tiles per core
SLOT_NKT = [8, 16, 24, 32]    # key tiles processed per slot (static)
K_REPL_BANDS = 2              # K bands 0..1 (tiles 0-15) replicated
V_REPL_BANDS = 2              # V bands 0..1 (tiles 0-15) replicated
K_GATH = 2                    # K contribution tiles per core ({16+c},{24+c})
V_GATH = 2                    # V contribution tiles per core ({16+c},{24+c})
K_KT = K_GATH + 8 * K_REPL_BANDS   # k input tiles per core (18)
V_KT = V_GATH + 8 * V_REPL_BANDS   # v input tiles per core (18)
VW = DH + 1                   # v columns per head incl. ones column (65)
EXP_BATCH = 8                 # psA tile width (x128) = 2 banks
F32 = mybir.dt.float32
BF16 = mybir.dt.bfloat16


def q_tiles_of_core(i):
    return [8 * j + i for j in range(N_SLOTS)]


def build(has_bias: bool):
    kt_in = 7 if has_bias else 6      # input-feature contraction tiles
    ho2 = 7 if has_bias else 6        # out-proj contraction chunks of 128

    nc = bacc.Bacc("TRN2", target_bir_lowering=False, debug=False,
                   num_devices=N_CORES)

    # ---- I/O ----
    qT_d = nc.dram_tensor("qT", [P, kt_in * NQ], BF16, kind="ExternalInput")
    # raw k rows: [contribution tiles {16+c},{24+c} | 16 repl tiles 0..15]
    kT_d = nc.dram_tensor("kT", [P, kt_in * K_KT * P], BF16,
                          kind="ExternalInput")
    # raw v rows: [2 contribution tiles {16+c},{24+c} | 16 repl tiles 0..15]
    vT_d = nc.dram_tensor("vT", [P, kt_in * V_KT * P], BF16,
                          kind="ExternalInput")
    wqT_d = nc.dram_tensor("wqT", [P, kt_in * D], BF16, kind="ExternalInput")
    wkT_d = nc.dram_tensor("wkT", [P, kt_in * D], BF16, kind="ExternalInput")
    wvT_d = nc.dram_tensor("wvT", [P, kt_in * D], BF16, kind="ExternalInput")
    # out-proj weight folded to [128, ho2, 768]: rows 64*par+d of chunk p
    # hold wo[:, (2p+par)*64+d]; bias chunk (if any) carries bo on row 0.
    wo2_d = nc.dram_tensor("wo2", [P, ho2 * D], BF16, kind="ExternalInput")
    # per-core causal mask over each slot's last 8 key tiles
    mask_d = nc.dram_tensor("mask", [P, 8 * P], BF16, kind="ExternalInput")
    ident_d = nc.dram_tensor("ident", [P, P], BF16, kind="ExternalInput")
    out_d = nc.dram_tensor("out", [NQ, D], F32, kind="ExternalOutput")

    # ---- combined all-gather bounce buffer [k2 | k3 | v2 | v3] ----
    cc = {}
    cc["kv", "in"] = nc.dram_tensor("cc_kv_in", [P, 24 * P], BF16)
    cc["kv", "out"] = nc.dram_tensor(
        "cc_kv_out", [N_CORES * P, 24 * P], BF16, addr_space="Shared")

    with tile.TileContext(nc) as tc:
        _body(nc, tc, locals(), has_bias, kt_in, ho2)

    nc.compile()
    return nc


def _body(nc, tc, t_, has_bias, kt_in, ho2):
    qT_d, kT_d, vT_d = t_["qT_d"], t_["kT_d"], t_["vT_d"]
    wqT_d, wkT_d, wvT_d, wo2_d = (t_["wqT_d"], t_["wkT_d"], t_["wvT_d"],
                                  t_["wo2_d"])
    mask_d, ident_d, out_d = t_["mask_d"], t_["ident_d"], t_["out_d"]
    cc = t_["cc"]

    from contextlib import ExitStack
    ctx = ExitStack()
    with ctx:
        # persistent tensors (live through attention)
        big = ctx.enter_context(tc.tile_pool(name="big", bufs=1))
        pw = ctx.enter_context(tc.tile_pool(name="pw", bufs=1))
        psA = ctx.enter_context(
            tc.tile_pool(name="psA", bufs=2, space="PSUM"))   # wide groups
        psPV = ctx.enter_context(
            tc.tile_pool(name="psPV", bufs=2, space="PSUM"))  # PV accum
        psM = ctx.enter_context(
            tc.tile_pool(name="psM", bufs=2, space="PSUM"))   # tp/outproj

        qhT = big.tile([P, 6, NQ], BF16, tag="qhT")
        khT = big.tile([P, 6, S], BF16, tag="khT")
        vh = big.tile([P, S // P, H, VW], BF16, tag="vh")
        aT2 = big.tile([P, ho2, NQ], BF16, tag="aT2")
        # partial PV accumulators for pair-1's kt<16 half (spilled from
        # PSUM so the kt>=16 half can run after the gather loads land)
        spill = big.tile([P, 6, 4 * VW], BF16, tag="spill")

        wk_sb = pw.tile([P, kt_in, D], BF16, tag="wk")
        wv_sb = pw.tile([P, kt_in, D], BF16, tag="wv")
        nc.scalar.dma_start(wk_sb[:], wkT_d.ap().rearrange(
            "p (kt f) -> p kt f", f=D))
        nc.scalar.dma_start(wv_sb[:], wvT_d.ap().rearrange(
            "p (kt f) -> p kt f", f=D))

        kT_v = kT_d.ap().rearrange("p (kt s) -> p kt s", s=K_KT * P)
        vT_v = vT_d.ap().rearrange("p (kt s) -> p kt s", s=V_KT * P)

        # prefetch both replicated V bands' raw rows up front: lazy
        # mid-attention fetches got semaphore-serialized behind the
        # collective timeline (a DMA wait implies completion of all
        # earlier-emitted DMAs on its semaphore)
        xvpre = [pw.tile([P, kt_in, 8 * P], BF16, tag="xv0", name="xv0"),
                 pw.tile([P, kt_in, 8 * P], BF16, tag="xv1", name="xv1")]
        for band in range(2):
            lo = (V_GATH + 8 * band) * P
            nc.scalar.dma_start(xvpre[band][:], vT_v[:, :, lo:lo + 8 * P])

        # ---------- contribution projections + gathers ----------
        with tc.tile_pool(name="ctr", bufs=1) as ctr:
            xkc = ctr.tile([P, kt_in, K_GATH * P], BF16, tag="xkc")
            nc.sync.dma_start(xkc[:], kT_v[:, :, :K_GATH * P])
            xvc = ctr.tile([P, kt_in, V_GATH * P], BF16, tag="xvc")
            nc.sync.dma_start(xvc[:], vT_v[:, :, :V_GATH * P])
            xq = ctr.tile([P, kt_in, NQ], BF16, tag="xq", bufs=1)
            nc.sync.dma_start(xq[:], qT_d.ap().rearrange(
                "p (kt s) -> p kt s", s=NQ))
            khT_c = ctr.tile([P, 6, K_GATH * P], BF16, tag="khT_c")
            vh_c = ctr.tile([P, V_GATH, 6 * P], BF16, tag="vh_c")

            # K contribution tiles {16+c}, {24+c}
            for ft in range(6):
                ps = psA.tile([P, EXP_BATCH * P], F32, tag="scores")
                for kt in range(kt_in):
                    nc.tensor.matmul(
                        ps[:, :K_GATH * P],
                        wk_sb[:, kt, ft * P:(ft + 1) * P],
                        xkc[:, kt, :],
                        start=(kt == 0), stop=(kt == kt_in - 1))
                nc.scalar.copy(khT_c[:, ft, :], ps[:, :K_GATH * P])
            cc_in_v = cc["kv", "in"].ap().rearrange(
                "p (b x) -> p b x", b=4)
            for b in range(K_GATH):
                nc.sync.dma_start(
                    cc_in_v[:, b, :].rearrange("p (ft w) -> p ft w", w=P),
                    khT_c[:, :, b * P:(b + 1) * P])

            # V contribution tiles {16+c}, {24+c}
            for b in range(V_GATH):
                for half in range(2):
                    ps = psA.tile([P, EXP_BATCH * P], F32, tag="scores")
                    for kt in range(kt_in):
                        nc.tensor.matmul(
                            ps[:, :384],
                            xvc[:, kt, b * P:(b + 1) * P],
                            wv_sb[:, kt, half * 384:(half + 1) * 384],
                            start=(kt == 0), stop=(kt == kt_in - 1))
                    nc.vector.tensor_copy(
                        vh_c[:, b, half * 384:(half + 1) * 384],
                        ps[:, :384])
                nc.sync.dma_start(cc_in_v[:, 2 + b, :], vh_c[:, b, :])

            # one combined gather: serial small gathers waste ~20us each
            nc.gpsimd.collective_compute(
                "AllGather", mybir.AluOpType.bypass,
                replica_groups=[list(range(N_CORES))],
                ins=[cc["kv", "in"].ap()],
                outs=[cc["kv", "out"].ap()],
            )

            # Q projection: weight streamed in two half-width pieces
            for piece in range(2):
                wq_sb = ctr.tile([P, kt_in, 384], BF16, tag="wq",
                                 name="wq_sb", bufs=2)
                nc.gpsimd.dma_start(
                    wq_sb[:],
                    wqT_d.ap().rearrange("p (kt f) -> p kt f", f=D)
                    [:, :, piece * 384:(piece + 1) * 384])
                for fh in range(3):
                    ft = piece * 3 + fh
                    ps = psA.tile([P, EXP_BATCH * P], F32, tag="scores")
                    for kt in range(kt_in):
                        nc.tensor.matmul(
                            ps[:, :NQ],
                            wq_sb[:, kt, fh * P:(fh + 1) * P],
                            xq[:, kt, :],
                            start=(kt == 0), stop=(kt == kt_in - 1))
                    nc.vector.tensor_copy(qhT[:, ft, :], ps[:, :NQ])

        nc.vector.memset(vh[:, :, :, DH:], 1.0)

        # gathered bands into SBUF; EMITTED at the pair-1 boundary: a DMA
        # wait implies completion of all earlier-emitted DMAs on the same
        # semaphore, so emitting these early would gate pair-0's fetches
        # on the collective.
        cc_out_v = cc["kv", "out"].ap().rearrange(
            "p (b x) -> p b x", b=4)
        def emit_gather_loads():
            for bi in range(2):
                for c in range(N_CORES):
                    gkt = 16 + 8 * bi + c
                    nc.sync.dma_start(
                        khT[:, :, gkt * P:(gkt + 1) * P],
                        cc_out_v[c * P:(c + 1) * P, bi, :].rearrange(
                            "p (ft w) -> p ft w", w=P))
            for bi in range(2):
                for c in range(N_CORES):
                    gkt = 16 + 8 * bi + c
                    nc.sync.dma_start(
                        vh[:, gkt, :, :DH],
                        cc_out_v[c * P:(c + 1) * P, 2 + bi, :].rearrange(
                            "p (hh w) -> p hh w", w=DH))

        # ---------- attention (pair-major) ----------
        if has_bias:
            # bias chunk of the out-projection: row 0 = 1 -> adds wo2 row 0
            nc.vector.memset(aT2[:, ho2 - 1, :], 0.0)
            nc.vector.memset(aT2[0:1, ho2 - 1, :], 1.0)

        with tc.tile_pool(name="expp", bufs=2) as expp, \
             tc.tile_pool(name="mskp", bufs=1) as mskp, \
             tc.tile_pool(name="smp", bufs=4) as smp, \
             tc.tile_pool(name="wop", bufs=1) as wop, \
             tc.tile_pool(name="xkp", bufs=2) as xkp, \
             tc.tile_pool(name="outp", bufs=1) as outp:
            tri = mskp.tile([P, 8 * P], BF16, tag="tri")
            nc.gpsimd.dma_start(tri[:], mask_d.ap())
            ident = mskp.tile([P, P], BF16, tag="ident")
            nc.gpsimd.dma_start(ident[:], ident_d.ap())
            wo_sb = wop.tile([P, ho2, D], BF16, tag="wo")
            nc.gpsimd.dma_start(wo_sb[:], wo2_d.ap().rearrange(
                "p (h f) -> p h f", f=D))

            def xk_fetch(band, hb):
                xk = xkp.tile([P, kt_in, 4 * P], BF16, tag="xk", name="xk")
                lo = (K_GATH + 8 * band + 4 * hb) * P
                nc.gpsimd.dma_start(xk[:], kT_v[:, :, lo:lo + 4 * P])
                return xk

            def k_repl(band, ft):
                # project K band tiles (8) for feature chunk ft into khT
                for hb in range(2):
                    xk = xk_fetch(band, hb)
                    ps = psA.tile([P, EXP_BATCH * P], F32, tag="scores")
                    for kt in range(kt_in):
                        nc.tensor.matmul(
                            ps[:, :4 * P],
                            wk_sb[:, kt, ft * P:(ft + 1) * P],
                            xk[:, kt, :],
                            start=(kt == 0), stop=(kt == kt_in - 1))
                    nc.vector.tensor_copy(
                        khT[:, ft, (8 * band + 4 * hb) * P:
                            (8 * band + 4 * hb + 4) * P],
                        ps[:, :4 * P])

            def v_repl(band, half):
                # project V band tiles (8), heads half*6..half*6+5, into vh
                for t in range(8):
                    ps = psA.tile([P, EXP_BATCH * P], F32, tag="scores")
                    for kt in range(kt_in):
                        nc.tensor.matmul(
                            ps[:, :384],
                            xvpre[band][:, kt, t * P:(t + 1) * P],
                            wv_sb[:, kt, half * 384:(half + 1) * 384],
                            start=(kt == 0), stop=(kt == kt_in - 1))
                    nc.vector.tensor_copy(
                        vh[:, 8 * band + t,
                           half * 6:(half + 1) * 6, :DH],
                        ps[:, :384].rearrange("p (hh w) -> p hh w",
                                              w=DH))

            # --- pair-major attention machinery ---
            # pair pr holds slots (2pr, 2pr+1); bands are 8-kt groups.
            # Within an ns=2 band, slot 2pr occupies cols 0:128 of each
            # 256-wide kt block and slot 2pr+1 cols 128:256; in an ns=1
            # band only slot 2pr+1 remains (cols 0:128).

            def band_ns(pr, band):
                return 2 if 8 * band < SLOT_NKT[2 * pr] else 1

            def scores_band(pr, hp, band, eb):
                """scores + exp into band tile eb [P, 2, 8*ns*128]."""
                ns = band_ns(pr, band)
                w = ns * P
                qoff = (2 * pr if ns == 2 else 2 * pr + 1) * P
                gsz = 4 // ns                 # kt per psum group
                for g in range(8 // gsz):
                    ps = psA.tile([P, 2 * EXP_BATCH // 2 * P], F32,
                                  tag="scores")
                    psv = ps[:].rearrange("p (hh g w) -> p hh g w",
                                          hh=2, g=gsz)
                    for i in range(gsz):
                        kt = 8 * band + g * gsz + i
                        nc.tensor.matmul(
                            psv[:, 0, i, :w],
                            khT[0:DH, hp, kt * P:(kt + 1) * P],
                            qhT[0:DH, hp, qoff:qoff + w],
                            start=True, stop=True)
                        nc.tensor.matmul(
                            psv[:, 1, i, :w],
                            khT[DH:P, hp, kt * P:(kt + 1) * P],
                            qhT[DH:P, hp, qoff:qoff + w],
                            start=True, stop=True)
                    nc.scalar.activation(
                        eb[:, :, g * gsz * w:(g + 1) * gsz * w],
                        ps[:].rearrange("p (hh c) -> p hh c", hh=2)
                        [:, :, :gsz * w],
                        mybir.ActivationFunctionType.Exp,
                        scale=0.125)
                # causal mask: band `band` is the diagonal band of slot
                # `band` globally; within this pair that slot (if present)
                # sits at cols 0:128 of each kt block.
                if band == 2 * pr or (ns == 1 and band == 2 * pr + 1):
                    ebv = eb[:].rearrange("p hh (kt w) -> p hh kt w", w=w)
                    for hh in range(2):
                        nc.vector.tensor_mul(
                            ebv[:, hh, :, 0:P],
                            ebv[:, hh, :, 0:P],
                            tri[:].rearrange("p (kt w) -> p kt w", w=P))

            def pv_band(pr, hp, band, eb, poT, bhi):
                """accumulate PV for one band into poT [P, 2, 2, VW]."""
                ns = band_ns(pr, band)
                w = ns * P
                ebv = eb[:].rearrange("p hh (kt w) -> p hh kt w", w=w)
                for i in range(8):
                    kt = 8 * band + i
                    for hh in range(2):
                        h = 2 * hp + hh
                        for si in range(2):
                            s = 2 * pr + si
                            if kt >= SLOT_NKT[s]:
                                continue
                            co = 0 if (ns == 1 or si == 0) else P
                            if ns == 1 and si == 0:
                                continue
                            # start=False always: a start=True wipes the
                            # whole 2KB PSUM bank, clobbering the other
                            # (hh, si) groups interleaved in this tile.
                            # poT is DVE-zeroed once per task instead.
                            nc.tensor.matmul(
                                poT[:, hh, si, :],
                                ebv[:, hh, i, co:co + P],
                                vh[:, kt, h, :],
                                start=False,
                                stop=(kt == min(SLOT_NKT[s], 8 * bhi) - 1))

            def finish_hp(pr, hp, poT, add_spill):
                """normalize poT, transpose into aT2 for both slots."""
                if add_spill:
                    nsum = smp.tile([P, 2, 2, VW], F32, tag="nsum")
                    nc.vector.tensor_add(
                        nsum[:], poT[:],
                        spill[:, hp, :].rearrange(
                            "p (hh si w) -> p hh si w", hh=2, si=2))
                    poT = nsum
                rec = smp.tile([P, 4], F32, tag="rec")
                for hh in range(2):
                    for si in range(2):
                        nc.vector.tensor_copy(
                            rec[:, 2 * hh + si:2 * hh + si + 1],
                            poT[:, hh, si, DH:DH + 1])
                nc.vector.reciprocal_approx_fast(rec[:, :], rec[:, :])
                for si in range(2):
                    s = 2 * pr + si
                    nrm = smp.tile([P, P], BF16, tag="nrm")
                    for hh in range(2):
                        nc.vector.tensor_scalar_mul(
                            nrm[:, hh * DH:(hh + 1) * DH],
                            poT[:, hh, si, 0:DH],
                            rec[:, 2 * hh + si:2 * hh + si + 1])
                    tp = psM.tile([P, P], BF16, tag="tp")
                    nc.tensor.transpose(tp[:, :], nrm[:, :], ident[:, :])
                    nc.vector.tensor_copy(aT2[:, hp, s * P:(s + 1) * P],
                                          tp[:, :P])

            def outproj(j):
                osb = outp.tile([P, D], F32, tag="osb")
                for half in range(2):
                    pw2 = psM.tile([P, 384], F32, tag="tp", name="pw2")
                    for p2 in range(ho2):
                        nc.tensor.matmul(
                            pw2[:, :384],
                            aT2[:, p2, j * P:(j + 1) * P],
                            wo_sb[:, p2, half * 384:(half + 1) * 384],
                            start=(p2 == 0), stop=(p2 == ho2 - 1))
                    nc.vector.tensor_copy(osb[:, half * 384:(half + 1) * 384],
                                          pw2[:, :384])
                nc.sync.dma_start(out_d[j * P:(j + 1) * P, :], osb[:])

            NBANDS = [2, 4]                  # bands per pair

            def hp_task(pr, hp, b0, b1, last):
                """head-pair task over bands b0..b1-1, band-pipelined.
                last=False spills the PV partials to SBUF instead of
                finishing (pair-1's kt<16 half)."""
                poT = psPV.tile([P, 2, 2, VW], F32, tag="poT")
                nc.vector.memset(poT[:], 0.0)
                ebs = {}
                for band in range(b0, b1):
                    ns = band_ns(pr, band)
                    eb = ebs[band] = expp.tile(
                        [P, 2, 8 * ns * P], BF16, tag="eb", name="eb")
                    scores_band(pr, hp, band, eb)
                    if band == b0 and pr == 0:
                        # pair-0 interleaves the replicated K/V projections
                        # between its first scores (whose exps keep the
                        # scalar engine busy) and the PV that consumes
                        # them: khT chunk for the NEXT head-pair, V halves
                        # when first needed (heads 0-5 at hp0, 6-11 at hp3)
                        if hp == 0:
                            v_repl(0, 0)
                            v_repl(1, 0)
                        if hp == 3:
                            v_repl(0, 1)
                            v_repl(1, 1)
                        if hp < 5:
                            k_repl(0, hp + 1)
                            k_repl(1, hp + 1)
                    if band - b0 >= 1:
                        pv_band(pr, hp, band - 1, ebs.pop(band - 1), poT,
                                b1)
                for band in sorted(ebs):
                    pv_band(pr, hp, band, ebs.pop(band), poT, b1)
                if last:
                    finish_hp(pr, hp, poT, add_spill=(b0 > 0))
                else:
                    nc.vector.tensor_copy(
                        spill[:, hp, :].rearrange(
                            "p (hh si w) -> p hh si w", hh=2, si=2),
                        poT[:])

            # schedule: pair-0 first (needs only replicated bands), with
            # pair-1 head-pairs (gathered bands, scalar-heavy) interleaved
            # once the all-gathers have had time to land.
            k_repl(0, 0)
            k_repl(1, 0)
            for hp in range(6):
                hp_task(0, hp, 0, 2, last=True)
            outproj(0)
            outproj(1)
            for hp in range(6):
                hp_task(1, hp, 0, 2, last=False)
            emit_gather_loads()
            for hp in range(6):
                hp_task(1, hp, 2, 4, last=True)
            outproj(2)
            outproj(3)


# ------------------------------------------------------------------
# host side
# ------------------------------------------------------------------

_CACHE = {}


def _get_nc(has_bias):
    key = has_bias
    if key not in _CACHE:
        _CACHE[key] = build(has_bias)
    return _CACHE[key]


def _bf16(x):
    return np.asarray(x, dtype=ml_dtypes.bfloat16)


def _build_mask(core):
    # Applied to the last 8 key-tiles of every slot (tiles 8j..8j+7, the
    # diagonal band): ones before this core's diagonal tile, transposed
    # lower-triangle at it (keep key_row <= q_col), zeros beyond.
    m = np.zeros((P, 8 * P), dtype=np.float32)
    m[:, :core * P] = 1.0
    m[:, core * P:(core + 1) * P] = np.tril(
        np.ones((P, P), dtype=np.float32)).T
    return _bf16(m)


def prepare_in_maps(q, k, v, wq, bq, wk, bk, wv, bv, wo, bo, has_bias):
    kt_in = 7 if has_bias else 6
    ho2 = 7 if has_bias else 6
    d_in = kt_in * P

    def _fold(x2d):
        # [kt_in*128, n] -> [128, kt_in*n] partition-major contiguous
        n = x2d.shape[1]
        return np.ascontiguousarray(
            x2d.reshape(kt_in, P, n).transpose(1, 0, 2).reshape(P, kt_in * n))

    def aug(xT, bias_row):
        # [768, n] -> folded [128, kt_in*n] with ones row at 768 (inputs)
        if not has_bias:
            return _bf16(_fold(xT))
        out = np.zeros((d_in, xT.shape[1]), dtype=np.float32)
        out[:D] = xT
        out[D] = bias_row
        return _bf16(_fold(out))

    def augw(w, b):
        # torch Linear weight [out, in] -> folded lhsT with bias row
        wT = w.T.astype(np.float32)
        if not has_bias:
            return _bf16(_fold(wT))
        out = np.zeros((d_in, D), dtype=np.float32)
        out[:D] = wT
        out[D] = b
        return _bf16(_fold(out))

    wqT = augw(wq, bq); wkT = augw(wk, bk); wvT = augw(wv, bv)

    # out-proj weight: chunk p rows 64*par+d = wo[:, (2p+par)*64+d]
    woT = wo.T.astype(np.float32)                       # [in, out]
    wo2 = np.zeros((P, ho2, D), dtype=np.float32)
    wo2[:, :6, :] = woT.reshape(6, 2, DH, D).transpose(1, 2, 0, 3).reshape(
        P, 6, D)
    if has_bias:
        wo2[0, 6, :] = bo
    wo2 = _bf16(np.ascontiguousarray(wo2.reshape(P, ho2 * D)))

    ident = _bf16(np.eye(P, dtype=np.float32))

    q2 = q[0].astype(np.float32)   # [S, D]
    k2 = k[0].astype(np.float32)
    v2 = v[0].astype(np.float32)

    in_maps = []
    for c in range(N_CORES):
        rows = np.concatenate(
            [np.arange(t * P, (t + 1) * P) for t in q_tiles_of_core(c)])
        k_tiles = [16 + c, 24 + c] + list(range(8 * K_REPL_BANDS))
        v_tiles = [16 + c, 24 + c] + list(range(8 * V_REPL_BANDS))
        krows = np.concatenate(
            [np.arange(t * P, (t + 1) * P) for t in k_tiles])
        vrows = np.concatenate(
            [np.arange(t * P, (t + 1) * P) for t in v_tiles])
        qT = aug(q2[rows].T, 1.0)
        kT = aug(k2[krows].T, 1.0)
        vT = aug(v2[vrows].T, 1.0)
        in_maps.append({
            "qT": qT, "kT": kT, "vT": vT,
            "wqT": wqT, "wkT": wkT, "wvT": wvT, "wo2": wo2,
            "mask": _build_mask(c), "ident": ident,
        })
    return in_maps


def kernel(q, k, v, wq, bq, wk, bk, wv, bv, wo, bo):
    q = np.asarray(q); k = np.asarray(k); v = np.asarray(v)
    wq = np.asarray(wq); wk = np.asarray(wk); wv = np.asarray(wv)
    wo = np.asarray(wo)
    bq = np.asarray(bq); bk = np.asarray(bk); bv = np.asarray(bv)
    bo = np.asarray(bo)
    has_bias = any(np.any(b) for b in (bq, bk, bv, bo))
    nc = _get_nc(has_bias)
    in_maps = prepare_in_maps(q, k, v, wq, bq, wk, bk, wv, bv, wo, bo,
                              has_bias)

    res = bass_utils.run_bass_kernel_spmd(
        nc, in_maps, core_ids=list(range(N_CORES)))
    kernel.last_exec_time_ns = res.exec_time_ns

    out = np.empty((S, D), dtype=np.float32)
    for c in range(N_CORES):
        for j, t in enumerate(q_tiles_of_core(c)):
            out[t * P:(t + 1) * P] = res.results[c]["out"][j * P:(j + 1) * P]
    return out.reshape(1, S, D)


# revision 21
# speedup vs baseline: 1.0616x; 1.0616x over previous
# Distributed causal multi-head attention for 8 TRN2 NeuronCores.
#
# Problem: B=1, S=4096, D=768, H=12 heads, d_head=64, fp32 I/O.
#   out = softmax(causal((x_q Wq^T)(x_k Wk^T)^T / 8)) (x_v Wv^T) Wo^T  (+biases)
#
# Sharding: sequence-parallel over queries.  The 32 q-tiles (128 rows each)
# are dealt to cores with stride 8: core i owns q-tiles {i, 8+i, 16+i, 24+i}
# ("slots" j=0..3).  To keep the graph SPMD-identical across cores, slot j
# statically processes n_j = 8j+8 key-tiles; keys beyond a core's causal
# limit are zeroed in the exp domain with a per-core 0/1 mask (data, not
# structure).
#
# Attention runs PAIR-major: pair 0 = slots (0,1) over key tiles 0-15,
# pair 1 = slots (2,3) over key tiles 0-31.  For key tiles where both
# slots of a pair are causally active, one scores matmul streams both
# slots' q columns against a single khT stationary load, halving the
# LDWEIGHTS traffic that dominated the slot-major version.  Scores land
# transposed ([keys, q]); exp output feeds the PV matmul as the stationary
# operand (streaming V's 65 columns incl. a ones-column that yields the
# softmax denominator per q partition).  Work is pipelined at band (8
# key-tile) granularity: scores(band+1) overlaps exp/mask/PV(band), and
# pair-1 head-pairs (scalar-engine heavy) are interleaved with pair-0's
# remaining projection work (PE heavy) to keep both engines fed.
#
# K/V distribution: collectives have a fixed ~58us startup before the
# first transfer can complete, so only the latest-needed bands go through
# them: K tiles 16-31 and V tiles 16-31 are sharded (core c projects tiles
# {16+c},{24+c}) and exchanged via four AllGathers ordered K2, V2, K3, V3
# to match first use.  K/V tiles 0-15 are projected redundantly per core,
# interleaved into the pair-0 head loop (K by feature chunk, V by head
# half).

import numpy as np
import ml_dtypes

import concourse.bass as bass
import concourse.mybir as mybir
import concourse.tile as tile
from concourse import bacc
from concourse import bass_utils

P = 128
S = 4096
D = 768
H = 12
DH = 64
N_CORES = 8
NQ = 512                      # q rows per core
N_SLOTS = 4                   # q-tiles per core
SLOT_NKT = [8, 16, 24, 32]    # key tiles processed per slot (static)
K_REPL_BANDS = 2              # K bands 0..1 (tiles 0-15) replicated
V_REPL_BANDS = 2              # V bands 0..1 (tiles 0-15) replicated
K_GATH = 2                    # K contribution tiles per core ({16+c},{24+c})
V_GATH = 2                    # V contribution tiles per core ({16+c},{24+c})
K_KT = K_GATH + 8 * K_REPL_BANDS   # k input tiles per core (18)
V_KT = V_GATH + 8 * V_REPL_BANDS   # v input tiles per core (18)
VW = DH + 1                   # v columns per head incl. ones column (65)
EXP_BATCH = 8                 # psA tile width (x128) = 2 banks
F32 = mybir.dt.float32
BF16 = mybir.dt.bfloat16


def q_tiles_of_core(i):
    return [8 * j + i for j in range(N_SLOTS)]


def build(has_bias: bool):
    kt_in = 7 if has_bias else 6      # input-feature contraction tiles
    ho2 = 7 if has_bias else 6        # out-proj contraction chunks of 128

    nc = bacc.Bacc("TRN2", target_bir_lowering=False, debug=False,
                   num_devices=N_CORES)

    # ---- I/O ----
    qT_d = nc.dram_tensor("qT", [P, kt_in * NQ], BF16, kind="ExternalInput")
    # raw k rows: [contribution tiles {16+c},{24+c} | 16 repl tiles 0..15]
    kT_d = nc.dram_tensor("kT", [P, kt_in * K_KT * P], BF16,
                          kind="ExternalInput")
    # raw v rows: [2 contribution tiles {16+c},{24+c} | 16 repl tiles 0..15]
    vT_d = nc.dram_tensor("vT", [P, kt_in * V_KT * P], BF16,
                          kind="ExternalInput")
    wqT_d = nc.dram_tensor("wqT", [P, kt_in * D], BF16, kind="ExternalInput")
    wkT_d = nc.dram_tensor("wkT", [P, kt_in * D], BF16, kind="ExternalInput")
    wvT_d = nc.dram_tensor("wvT", [P, kt_in * D], BF16, kind="ExternalInput")
    # out-proj weight folded to [128, ho2, 768]: rows 64*par+d of chunk p
    # hold wo[:, (2p+par)*64+d]; bias chunk (if any) carries bo on row 0.
    wo2_d = nc.dram_tensor("wo2", [P, ho2 * D], BF16, kind="ExternalInput")
    # per-core causal mask over each slot's last 8 key tiles
    mask_d = nc.dram_tensor("mask", [P, 8 * P], BF16, kind="ExternalInput")
    ident_d = nc.dram_tensor("ident", [P, P], BF16, kind="ExternalInput")
    out_d = nc.dram_tensor("out", [NQ, D], F32, kind="ExternalOutput")

    # ---- combined all-gather bounce buffer [k2 | k3 | v2 | v3] ----
    cc = {}
    cc["kv", "in"] = nc.dram_tensor("cc_kv_in", [P, 24 * P], BF16)
    cc["kv", "out"] = nc.dram_tensor(
        "cc_kv_out", [N_CORES * P, 24 * P], BF16, addr_space="Shared")

    with tile.TileContext(nc) as tc:
        _body(nc, tc, locals(), has_bias, kt_in, ho2)

    nc.compile()
    return nc


def _body(nc, tc, t_, has_bias, kt_in, ho2):
    qT_d, kT_d, vT_d = t_["qT_d"], t_["kT_d"], t_["vT_d"]
    wqT_d, wkT_d, wvT_d, wo2_d = (t_["wqT_d"], t_["wkT_d"], t_["wvT_d"],
                                  t_["wo2_d"])
    mask_d, ident_d, out_d = t_["mask_d"], t_["ident_d"], t_["out_d"]
    cc = t_["cc"]

    from contextlib import ExitStack
    ctx = ExitStack()
    with ctx:
        # persistent tensors (live through attention)
        big = ctx.enter_context(tc.tile_pool(name="big", bufs=1))
        pw = ctx.enter_context(tc.tile_pool(name="pw", bufs=1))
        psA = ctx.enter_context(
            tc.tile_pool(name="psA", bufs=2, space="PSUM"))   # wide groups
        psPV = ctx.enter_context(
            tc.tile_pool(name="psPV", bufs=2, space="PSUM"))  # PV accum
        psM = ctx.enter_context(
            tc.tile_pool(name="psM", bufs=2, space="PSUM"))   # tp/outproj

        qhT = big.tile([P, 6, NQ], BF16, tag="qhT")
        khT = big.tile([P, 6, S], BF16, tag="khT")
        vh = big.tile([P, S // P, H, VW], BF16, tag="vh")
        aT2 = big.tile([P, ho2, NQ], BF16, tag="aT2")
        # partial PV accumulators for pair-1's kt<16 half (spilled from
        # PSUM so the kt>=16 half can run after the gather loads land)
        spill = big.tile([P, 6, 4 * VW], BF16, tag="spill")

        wk_sb = pw.tile([P, kt_in, D], BF16, tag="wk")
        wv_sb = pw.tile([P, kt_in, D], BF16, tag="wv")
        nc.scalar.dma_start(wk_sb[:], wkT_d.ap().rearrange(
            "p (kt f) -> p kt f", f=D))
        nc.scalar.dma_start(wv_sb[:], wvT_d.ap().rearrange(
            "p (kt f) -> p kt f", f=D))

        kT_v = kT_d.ap().rearrange("p (kt s) -> p kt s", s=K_KT * P)
        vT_v = vT_d.ap().rearrange("p (kt s) -> p kt s", s=V_KT * P)

        # prefetch both replicated V bands' raw rows up front: lazy
        # mid-attention fetches got semaphore-serialized behind the
        # collective timeline (a DMA wait implies completion of all
        # earlier-emitted DMAs on its semaphore)
        xvpre = [pw.tile([P, kt_in, 8 * P], BF16, tag="xv0", name="xv0"),
                 pw.tile([P, kt_in, 8 * P], BF16, tag="xv1", name="xv1")]

        # ---------- contribution projections + gathers ----------
        with tc.tile_pool(name="ctr", bufs=1) as ctr:
            xkc = ctr.tile([P, kt_in, K_GATH * P], BF16, tag="xkc")
            nc.sync.dma_start(xkc[:], kT_v[:, :, :K_GATH * P])
            xvc = ctr.tile([P, kt_in, V_GATH * P], BF16, tag="xvc")
            nc.sync.dma_start(xvc[:], vT_v[:, :, :V_GATH * P])
            xq = ctr.tile([P, kt_in, NQ], BF16, tag="xq", bufs=1)
            nc.sync.dma_start(xq[:], qT_d.ap().rearrange(
                "p (kt s) -> p kt s", s=NQ))
            for band in range(2):
                lo = (V_GATH + 8 * band) * P
                nc.sync.dma_start(xvpre[band][:],
                                  vT_v[:, :, lo:lo + 8 * P])
            khT_c = ctr.tile([P, 6, K_GATH * P], BF16, tag="khT_c")
            vh_c = ctr.tile([P, V_GATH, 6 * P], BF16, tag="vh_c")

            # K contribution tiles {16+c}, {24+c}
            for ft in range(6):
                ps = psA.tile([P, EXP_BATCH * P], F32, tag="scores")
                for kt in range(kt_in):
                    nc.tensor.matmul(
                        ps[:, :K_GATH * P],
                        wk_sb[:, kt, ft * P:(ft + 1) * P],
                        xkc[:, kt, :],
                        start=(kt == 0), stop=(kt == kt_in - 1))
                nc.scalar.copy(khT_c[:, ft, :], ps[:, :K_GATH * P])
            cc_in_v = cc["kv", "in"].ap().rearrange(
                "p (b x) -> p b x", b=4)
            for b in range(K_GATH):
                nc.sync.dma_start(
                    cc_in_v[:, b, :].rearrange("p (ft w) -> p ft w", w=P),
                    khT_c[:, :, b * P:(b + 1) * P])

            # V contribution tiles {16+c}, {24+c}
            for b in range(V_GATH):
                for half in range(2):
                    ps = psA.tile([P, EXP_BATCH * P], F32, tag="scores")
                    for kt in range(kt_in):
                        nc.tensor.matmul(
                            ps[:, :384],
                            xvc[:, kt, b * P:(b + 1) * P],
                            wv_sb[:, kt, half * 384:(half + 1) * 384],
                            start=(kt == 0), stop=(kt == kt_in - 1))
                    nc.vector.tensor_copy(
                        vh_c[:, b, half * 384:(half + 1) * 384],
                        ps[:, :384])
                nc.sync.dma_start(cc_in_v[:, 2 + b, :], vh_c[:, b, :])

            # one combined gather: serial small gathers waste ~20us each
            nc.gpsimd.collective_compute(
                "AllGather", mybir.AluOpType.bypass,
                replica_groups=[list(range(N_CORES))],
                ins=[cc["kv", "in"].ap()],
                outs=[cc["kv", "out"].ap()],
            )

            # Q projection: weight streamed in two half-width pieces
            for piece in range(2):
                wq_sb = ctr.tile([P, kt_in, 384], BF16, tag="wq",
                                 name="wq_sb", bufs=2)
                nc.gpsimd.dma_start(
                    wq_sb[:],
                    wqT_d.ap().rearrange("p (kt f) -> p kt f", f=D)
                    [:, :, piece * 384:(piece + 1) * 384])
                for fh in range(3):
                    ft = piece * 3 + fh
                    ps = psA.tile([P, EXP_BATCH * P], F32, tag="scores")
                    for kt in range(kt_in):
                        nc.tensor.matmul(
                            ps[:, :NQ],
                            wq_sb[:, kt, fh * P:(fh + 1) * P],
                            xq[:, kt, :],
                            start=(kt == 0), stop=(kt == kt_in - 1))
                    nc.vector.tensor_copy(qhT[:, ft, :], ps[:, :NQ])

        nc.vector.memset(vh[:, :, :, DH:], 1.0)

        # gathered bands into SBUF; EMITTED at the pair-1 boundary: a DMA
        # wait implies completion of all earlier-emitted DMAs on the same
        # semaphore, so emitting these early would gate pair-0's fetches
        # on the collective.
        cc_out_v = cc["kv", "out"].ap().rearrange(
            "p (b x) -> p b x", b=4)
        def emit_gather_loads():
            for bi in range(2):
                for c in range(N_CORES):
                    gkt = 16 + 8 * bi + c
                    nc.sync.dma_start(
                        khT[:, :, gkt * P:(gkt + 1) * P],
                        cc_out_v[c * P:(c + 1) * P, bi, :].rearrange(
                            "p (ft w) -> p ft w", w=P))
            for bi in range(2):
                for c in range(N_CORES):
                    gkt = 16 + 8 * bi + c
                    nc.sync.dma_start(
                        vh[:, gkt, :, :DH],
                        cc_out_v[c * P:(c + 1) * P, 2 + bi, :].rearrange(
                            "p (hh w) -> p hh w", w=DH))

        # ---------- attention (pair-major) ----------
        if has_bias:
            # bias chunk of the out-projection: row 0 = 1 -> adds wo2 row 0
            nc.vector.memset(aT2[:, ho2 - 1, :], 0.0)
            nc.vector.memset(aT2[0:1, ho2 - 1, :], 1.0)

        with tc.tile_pool(name="expp", bufs=2) as expp, \
             tc.tile_pool(name="mskp", bufs=1) as mskp, \
             tc.tile_pool(name="smp", bufs=4) as smp, \
             tc.tile_pool(name="wop", bufs=1) as wop, \
             tc.tile_pool(name="xkp", bufs=2) as xkp, \
             tc.tile_pool(name="outp", bufs=1) as outp:
            tri = mskp.tile([P, 8 * P], BF16, tag="tri")
            nc.gpsimd.dma_start(tri[:], mask_d.ap())
            ident = mskp.tile([P, P], BF16, tag="ident")
            nc.gpsimd.dma_start(ident[:], ident_d.ap())
            wo_sb = wop.tile([P, ho2, D], BF16, tag="wo")
            nc.gpsimd.dma_start(wo_sb[:], wo2_d.ap().rearrange(
                "p (h f) -> p h f", f=D))

            def xk_fetch(band, hb):
                xk = xkp.tile([P, kt_in, 4 * P], BF16, tag="xk", name="xk")
                lo = (K_GATH + 8 * band + 4 * hb) * P
                nc.gpsimd.dma_start(xk[:], kT_v[:, :, lo:lo + 4 * P])
                return xk

            def k_repl(band, ft):
                # project K band tiles (8) for feature chunk ft into khT
                for hb in range(2):
                    xk = xk_fetch(band, hb)
                    ps = psA.tile([P, EXP_BATCH * P], F32, tag="scores")
                    for kt in range(kt_in):
                        nc.tensor.matmul(
                            ps[:, :4 * P],
                            wk_sb[:, kt, ft * P:(ft + 1) * P],
                            xk[:, kt, :],
                            start=(kt == 0), stop=(kt == kt_in - 1))
                    nc.vector.tensor_copy(
                        khT[:, ft, (8 * band + 4 * hb) * P:
                            (8 * band + 4 * hb + 4) * P],
                        ps[:, :4 * P])

            def v_repl(band, half):
                # project V band tiles (8), heads half*6..half*6+5, into vh
                for t in range(8):
                    ps = psA.tile([P, EXP_BATCH * P], F32, tag="scores")
                    for kt in range(kt_in):
                        nc.tensor.matmul(
                            ps[:, :384],
                            xvpre[band][:, kt, t * P:(t + 1) * P],
                            wv_sb[:, kt, half * 384:(half + 1) * 384],
                            start=(kt == 0), stop=(kt == kt_in - 1))
                    nc.vector.tensor_copy(
                        vh[:, 8 * band + t,
                           half * 6:(half + 1) * 6, :DH],
                        ps[:, :384].rearrange("p (hh w) -> p hh w",
                                              w=DH))

            # --- pair-major attention machinery ---
            # pair pr holds slots (2pr, 2pr+1); bands are 8-kt groups.
            # Within an ns=2 band, slot 2pr occupies cols 0:128 of each
            # 256-wide kt block and slot 2pr+1 cols 128:256; in an ns=1
            # band only slot 2pr+1 remains (cols 0:128).

            def band_ns(pr, band):
                return 2 if 8 * band < SLOT_NKT[2 * pr] else 1

            def scores_band(pr, hp, band, eb):
                """scores + exp into band tile eb [P, 2, 8*ns*128]."""
                ns = band_ns(pr, band)
                w = ns * P
                qoff = (2 * pr if ns == 2 else 2 * pr + 1) * P
                gsz = 4 // ns                 # kt per psum group
                for g in range(8 // gsz):
                    ps = psA.tile([P, 2 * EXP_BATCH // 2 * P], F32,
                                  tag="scores")
                    psv = ps[:].rearrange("p (hh g w) -> p hh g w",
                                          hh=2, g=gsz)
                    for i in range(gsz):
                        kt = 8 * band + g * gsz + i
                        nc.tensor.matmul(
                            psv[:, 0, i, :w],
                            khT[0:DH, hp, kt * P:(kt + 1) * P],
                            qhT[0:DH, hp, qoff:qoff + w],
                            start=True, stop=True)
                        nc.tensor.matmul(
                            psv[:, 1, i, :w],
                            khT[DH:P, hp, kt * P:(kt + 1) * P],
                            qhT[DH:P, hp, qoff:qoff + w],
                            start=True, stop=True)
                    nc.scalar.activation(
                        eb[:, :, g * gsz * w:(g + 1) * gsz * w],
                        ps[:].rearrange("p (hh c) -> p hh c", hh=2)
                        [:, :, :gsz * w],
                        mybir.ActivationFunctionType.Exp,
                        scale=0.125)
                # causal mask: band `band` is the diagonal band of slot
                # `band` globally; within this pair that slot (if present)
                # sits at cols 0:128 of each kt block.
                if band == 2 * pr or (ns == 1 and band == 2 * pr + 1):
                    ebv = eb[:].rearrange("p hh (kt w) -> p hh kt w", w=w)
                    for hh in range(2):
                        nc.vector.tensor_mul(
                            ebv[:, hh, :, 0:P],
                            ebv[:, hh, :, 0:P],
                            tri[:].rearrange("p (kt w) -> p kt w", w=P))

            def pv_band(pr, hp, band, eb, poT, bhi):
                """accumulate PV for one band into poT [P, 2, 2, VW]."""
                ns = band_ns(pr, band)
                w = ns * P
                ebv = eb[:].rearrange("p hh (kt w) -> p hh kt w", w=w)
                for i in range(8):
                    kt = 8 * band + i
                    for hh in range(2):
                        h = 2 * hp + hh
                        for si in range(2):
                            s = 2 * pr + si
                            if kt >= SLOT_NKT[s]:
                                continue
                            co = 0 if (ns == 1 or si == 0) else P
                            if ns == 1 and si == 0:
                                continue
                            # start=False always: a start=True wipes the
                            # whole 2KB PSUM bank, clobbering the other
                            # (hh, si) groups interleaved in this tile.
                            # poT is DVE-zeroed once per task instead.
                            nc.tensor.matmul(
                                poT[:, hh, si, :],
                                ebv[:, hh, i, co:co + P],
                                vh[:, kt, h, :],
                                start=False,
                                stop=(kt == min(SLOT_NKT[s], 8 * bhi) - 1))

            def finish_hp(pr, hp, poT, add_spill):
                """normalize poT, transpose into aT2 for both slots."""
                if add_spill:
                    nsum = smp.tile([P, 2, 2, VW], F32, tag="nsum")
                    nc.vector.tensor_add(
                        nsum[:], poT[:],
                        spill[:, hp, :].rearrange(
                            "p (hh si w) -> p hh si w", hh=2, si=2))
                    poT = nsum
                rec = smp.tile([P, 4], F32, tag="rec")
                for hh in range(2):
                    for si in range(2):
                        nc.vector.tensor_copy(
                            rec[:, 2 * hh + si:2 * hh + si + 1],
                            poT[:, hh, si, DH:DH + 1])
                nc.vector.reciprocal_approx_fast(rec[:, :], rec[:, :])
                for si in range(2):
                    s = 2 * pr + si
                    nrm = smp.tile([P, P], BF16, tag="nrm")
                    for hh in range(2):
                        nc.vector.tensor_scalar_mul(
                            nrm[:, hh * DH:(hh + 1) * DH],
                            poT[:, hh, si, 0:DH],
                            rec[:, 2 * hh + si:2 * hh + si + 1])
                    tp = psM.tile([P, P], BF16, tag="tp")
                    nc.tensor.transpose(tp[:, :], nrm[:, :], ident[:, :])
                    nc.vector.tensor_copy(aT2[:, hp, s * P:(s + 1) * P],
                                          tp[:, :P])

            def outproj(j):
                osb = outp.tile([P, D], F32, tag="osb")
                for half in range(2):
                    pw2 = psM.tile([P, 384], F32, tag="tp", name="pw2")
                    for p2 in range(ho2):
                        nc.tensor.matmul(
                            pw2[:, :384],
                            aT2[:, p2, j * P:(j + 1) * P],
                            wo_sb[:, p2, half * 384:(half + 1) * 384],
                            start=(p2 == 0), stop=(p2 == ho2 - 1))
                    nc.vector.tensor_copy(osb[:, half * 384:(half + 1) * 384],
                                          pw2[:, :384])
                nc.sync.dma_start(out_d[j * P:(j + 1) * P, :], osb[:])

            NBANDS = [2, 4]                  # bands per pair

            def hp_task(pr, hp, b0, b1, last):
                """head-pair task over bands b0..b1-1, band-pipelined.
                last=False spills the PV partials to SBUF instead of
                finishing (pair-1's kt<16 half)."""
                poT = psPV.tile([P, 2, 2, VW], F32, tag="poT")
                nc.vector.memset(poT[:], 0.0)
                ebs = {}
                for band in range(b0, b1):
                    ns = band_ns(pr, band)
                    eb = ebs[band] = expp.tile(
                        [P, 2, 8 * ns * P], BF16, tag="eb", name="eb")
                    scores_band(pr, hp, band, eb)
                    if band == b0 and pr == 0:
                        # pair-0 interleaves the replicated K/V projections
                        # between its first scores (whose exps keep the
                        # scalar engine busy) and the PV that consumes
                        # them: khT chunk for the NEXT head-pair, V halves
                        # when first needed (heads 0-5 at hp0, 6-11 at hp3)
                        if hp == 0:
                            v_repl(0, 0)
                            v_repl(1, 0)
                        if hp == 3:
                            v_repl(0, 1)
                            v_repl(1, 1)
                        if hp < 5:
                            k_repl(0, hp + 1)
                            k_repl(1, hp + 1)
                    if band - b0 >= 1:
                        pv_band(pr, hp, band - 1, ebs.pop(band - 1), poT,
                                b1)
                for band in sorted(ebs):
                    pv_band(pr, hp, band, ebs.pop(band), poT, b1)
                if last:
                    finish_hp(pr, hp, poT, add_spill=(b0 > 0))
                else:
                    nc.vector.tensor_copy(
                        spill[:, hp, :].rearrange(
                            "p (hh si w) -> p hh si w", hh=2, si=2),
                        poT[:])

            # schedule: pair-0 first (needs only replicated bands), with
            # pair-1 head-pairs (gathered bands, scalar-heavy) interleaved
            # once the all-gathers have had time to land.
            k_repl(0, 0)
            k_repl(1, 0)
            seq = [(0, 0), (0, 1), (1, 0), (0, 2), (1, 1), (0, 3),
                   (1, 2), (0, 4), (1, 3), (0, 5), (1, 4), (1, 5)]
            for pr, hp in seq:
                if pr == 0:
                    hp_task(0, hp, 0, 2, last=True)
                else:
                    hp_task(1, hp, 0, 2, last=False)
            outproj(0)
            outproj(1)
            emit_gather_loads()
            for hp in range(6):
                hp_task(1, hp, 2, 4, last=True)
            outproj(2)
            outproj(3)


# ------------------------------------------------------------------
# host side
# ------------------------------------------------------------------

_CACHE = {}


def _get_nc(has_bias):
    key = has_bias
    if key not in _CACHE:
        _CACHE[key] = build(has_bias)
    return _CACHE[key]


def _bf16(x):
    return np.asarray(x, dtype=ml_dtypes.bfloat16)


def _build_mask(core):
    # Applied to the last 8 key-tiles of every slot (tiles 8j..8j+7, the
    # diagonal band): ones before this core's diagonal tile, transposed
    # lower-triangle at it (keep key_row <= q_col), zeros beyond.
    m = np.zeros((P, 8 * P), dtype=np.float32)
    m[:, :core * P] = 1.0
    m[:, core * P:(core + 1) * P] = np.tril(
        np.ones((P, P), dtype=np.float32)).T
    return _bf16(m)


def prepare_in_maps(q, k, v, wq, bq, wk, bk, wv, bv, wo, bo, has_bias):
    kt_in = 7 if has_bias else 6
    ho2 = 7 if has_bias else 6
    d_in = kt_in * P

    def _fold(x2d):
        # [kt_in*128, n] -> [128, kt_in*n] partition-major contiguous
        n = x2d.shape[1]
        return np.ascontiguousarray(
            x2d.reshape(kt_in, P, n).transpose(1, 0, 2).reshape(P, kt_in * n))

    def aug(xT, bias_row):
        # [768, n] -> folded [128, kt_in*n] with ones row at 768 (inputs)
        if not has_bias:
            return _bf16(_fold(xT))
        out = np.zeros((d_in, xT.shape[1]), dtype=np.float32)
        out[:D] = xT
        out[D] = bias_row
        return _bf16(_fold(out))

    def augw(w, b):
        # torch Linear weight [out, in] -> folded lhsT with bias row
        wT = w.T.astype(np.float32)
        if not has_bias:
            return _bf16(_fold(wT))
        out = np.zeros((d_in, D), dtype=np.float32)
        out[:D] = wT
        out[D] = b
        return _bf16(_fold(out))

    wqT = augw(wq, bq); wkT = augw(wk, bk); wvT = augw(wv, bv)

    # out-proj weight: chunk p rows 64*par+d = wo[:, (2p+par)*64+d]
    woT = wo.T.astype(np.float32)                       # [in, out]
    wo2 = np.zeros((P, ho2, D), dtype=np.float32)
    wo2[:, :6, :] = woT.reshape(6, 2, DH, D).transpose(1, 2, 0, 3).reshape(
        P, 6, D)
    if has_bias:
        wo2[0, 6, :] = bo
    wo2 = _bf16(np.ascontiguousarray(wo2.reshape(P, ho2 * D)))

    ident = _bf16(np.eye(P, dtype=np.float32))

    q2 = q[0].astype(np.float32)   # [S, D]
    k2 = k[0].astype(np.float32)
    v2 = v[0].astype(np.float32)

    in_maps = []
    for c in range(N_CORES):
        rows = np.concatenate(
            [np.arange(t * P, (t + 1) * P) for t in q_tiles_of_core(c)])
        k_tiles = [16 + c, 24 + c] + list(range(8 * K_REPL_BANDS))
        v_tiles = [16 + c, 24 + c] + list(range(8 * V_REPL_BANDS))
        krows = np.concatenate(
            [np.arange(t * P, (t + 1) * P) for t in k_tiles])
        vrows = np.concatenate(
            [np.arange(t * P, (t + 1) * P) for t in v_tiles])
        qT = aug(q2[rows].T, 1.0)
        kT = aug(k2[krows].T, 1.0)
        vT = aug(v2[vrows].T, 1.0)
        in_maps.append({
            "qT": qT, "kT": kT, "vT": vT,
            "wqT": wqT, "wkT": wkT, "wvT": wvT, "wo2": wo2,
            "mask": _build_mask(c), "ident": ident,
        })
    return in_maps


def kernel(q, k, v, wq, bq, wk, bk, wv, bv, wo, bo):
    q = np.asarray(q); k = np.asarray(k); v = np.asarray(v)
    wq = np.asarray(wq); wk = np.asarray(wk); wv = np.asarray(wv)
    wo = np.asarray(wo)
    bq = np.asarray(bq); bk = np.asarray(bk); bv = np.asarray(bv)
    bo = np.asarray(bo)
    has_bias = any(np.any(b) for b in (bq, bk, bv, bo))
    nc = _get_nc(has_bias)
    in_maps = prepare_in_maps(q, k, v, wq, bq, wk, bk, wv, bv, wo, bo,
                              has_bias)

    res = bass_utils.run_bass_kernel_spmd(
        nc, in_maps, core_ids=list(range(N_CORES)))
    kernel.last_exec_time_ns = res.exec_time_ns

    out = np.empty((S, D), dtype=np.float32)
    for c in range(N_CORES):
        for j, t in enumerate(q_tiles_of_core(c)):
            out[t * P:(t + 1) * P] = res.results[c]["out"][j * P:(j + 1) * P]
    return out.reshape(1, S, D)


# revision 22
# speedup vs baseline: 1.0619x; 1.0003x over previous
# Distributed causal multi-head attention for 8 TRN2 NeuronCores.
#
# Problem: B=1, S=4096, D=768, H=12 heads, d_head=64, fp32 I/O.
#   out = softmax(causal((x_q Wq^T)(x_k Wk^T)^T / 8)) (x_v Wv^T) Wo^T  (+biases)
#
# Sharding: sequence-parallel over queries.  The 32 q-tiles (128 rows each)
# are dealt to cores with stride 8: core i owns q-tiles {i, 8+i, 16+i, 24+i}
# ("slots" j=0..3).  To keep the graph SPMD-identical across cores, slot j
# statically processes n_j = 8j+8 key-tiles; keys beyond a core's causal
# limit are zeroed in the exp domain with a per-core 0/1 mask (data, not
# structure).
#
# K/V distribution: collectives have a ~65us first-op latency plus ~20us
# per op on this stack, so only the latest-needed bands go through them:
# K tiles 16-31 and V tiles 16-31 are sharded (core c projects tiles
# {16+c},{24+c} of K and V) and exchanged via four AllGathers.
# Everything earlier (K 0-15, V 0-15) is projected redundantly on every
# core, interleaved into the attention loop at (band, head-pair)
# granularity so scores unlock as soon as their feature chunk lands.
#
# Attention layout: scores land transposed ([keys, q]) so exp output feeds
# the PV matmul as the stationary operand (streaming only V's 65 columns,
# incl. a ones-column that yields the softmax denominator per q partition).
# PV output is [q, feat]; a per-partition reciprocal multiply normalizes it,
# a PE transpose flips each head-pair block to [feat, q], and the output
# projection contracts 128 features at a time.

import numpy as np
import ml_dtypes

import concourse.bass as bass
import concourse.mybir as mybir
import concourse.tile as tile
from concourse import bacc
from concourse import bass_utils

P = 128
S = 4096
D = 768
H = 12
DH = 64
N_CORES = 8
NQ = 512                      # q rows per core
N_SLOTS = 4                   # q-tiles per core
SLOT_NKT = [8, 16, 24, 32]    # key tiles processed per slot (static)
K_REPL_BANDS = 2              # K bands 0..1 (tiles 0-15) replicated
V_REPL_BANDS = 2              # V bands 0..1 (tiles 0-15) replicated
K_GATH = 2                    # K contribution tiles per core ({16+c},{24+c})
V_GATH = 2                    # V contribution tiles per core ({16+c},{24+c})
K_KT = K_GATH + 8 * K_REPL_BANDS   # k input tiles per core (18)
V_KT = V_GATH + 8 * V_REPL_BANDS   # v input tiles per core (18)
VW = DH + 1                   # v columns per head incl. ones column (65)
EXP_BATCH = 8                 # psA tile width (x128) = 2 banks
F32 = mybir.dt.float32
BF16 = mybir.dt.bfloat16


def q_tiles_of_core(i):
    return [8 * j + i for j in range(N_SLOTS)]


def build(has_bias: bool):
    kt_in = 7 if has_bias else 6      # input-feature contraction tiles
    ho2 = 7 if has_bias else 6        # out-proj contraction chunks of 128

    nc = bacc.Bacc("TRN2", target_bir_lowering=False, debug=False,
                   num_devices=N_CORES)

    # ---- I/O ----
    qT_d = nc.dram_tensor("qT", [P, kt_in * NQ], BF16, kind="ExternalInput")
    # raw k rows: [contribution tiles {16+c},{24+c} | 16 repl tiles 0..15]
    kT_d = nc.dram_tensor("kT", [P, kt_in * K_KT * P], BF16,
                          kind="ExternalInput")
    # raw v rows: [2 contribution tiles {16+c},{24+c} | 16 repl tiles 0..15]
    vT_d = nc.dram_tensor("vT", [P, kt_in * V_KT * P], BF16,
                          kind="ExternalInput")
    wqT_d = nc.dram_tensor("wqT", [P, kt_in * D], BF16, kind="ExternalInput")
    wkT_d = nc.dram_tensor("wkT", [P, kt_in * D], BF16, kind="ExternalInput")
    wvT_d = nc.dram_tensor("wvT", [P, kt_in * D], BF16, kind="ExternalInput")
    # out-proj weight folded to [128, ho2, 768]: rows 64*par+d of chunk p
    # hold wo[:, (2p+par)*64+d]; bias chunk (if any) carries bo on row 0.
    wo2_d = nc.dram_tensor("wo2", [P, ho2 * D], BF16, kind="ExternalInput")
    # per-core causal mask over each slot's last 8 key tiles
    mask_d = nc.dram_tensor("mask", [P, 8 * P], BF16, kind="ExternalInput")
    ident_d = nc.dram_tensor("ident", [P, P], BF16, kind="ExternalInput")
    out_d = nc.dram_tensor("out", [NQ, D], F32, kind="ExternalOutput")

    # ---- banded all-gather bounce buffers ----
    cc = {}
    for name in ("k2", "k3", "v2", "v3"):
        cc[name, "in"] = nc.dram_tensor(f"cc_{name}_in", [P, 6 * P], BF16)
        cc[name, "out"] = nc.dram_tensor(
            f"cc_{name}_out", [N_CORES * P, 6 * P], BF16,
            addr_space="Shared")

    with tile.TileContext(nc) as tc:
        _body(nc, tc, locals(), has_bias, kt_in, ho2)

    nc.compile()
    return nc


def _body(nc, tc, t_, has_bias, kt_in, ho2):
    qT_d, kT_d, vT_d = t_["qT_d"], t_["kT_d"], t_["vT_d"]
    wqT_d, wkT_d, wvT_d, wo2_d = (t_["wqT_d"], t_["wkT_d"], t_["wvT_d"],
                                  t_["wo2_d"])
    mask_d, ident_d, out_d = t_["mask_d"], t_["ident_d"], t_["out_d"]
    cc = t_["cc"]

    from contextlib import ExitStack
    ctx = ExitStack()
    with ctx:
        # persistent tensors (live through attention)
        big = ctx.enter_context(tc.tile_pool(name="big", bufs=1))
        pw = ctx.enter_context(tc.tile_pool(name="pw", bufs=1))
        st = ctx.enter_context(tc.tile_pool(name="st", bufs=1))
        psA = ctx.enter_context(
            tc.tile_pool(name="psA", bufs=2, space="PSUM"))   # wide groups
        psB = ctx.enter_context(
            tc.tile_pool(name="psB", bufs=4, space="PSUM"))   # PV/tp/outproj

        qhT = big.tile([P, 6, NQ], BF16, tag="qhT")
        khT = big.tile([P, 6, S], BF16, tag="khT")
        vh = big.tile([P, S // P, H, VW], BF16, tag="vh")
        aT2 = big.tile([P, ho2, NQ], BF16, tag="aT2")

        wk_sb = pw.tile([P, kt_in, D], BF16, tag="wk")
        wv_sb = pw.tile([P, kt_in, D], BF16, tag="wv")
        nc.scalar.dma_start(wk_sb[:], wkT_d.ap().rearrange(
            "p (kt f) -> p kt f", f=D))
        nc.scalar.dma_start(wv_sb[:], wvT_d.ap().rearrange(
            "p (kt f) -> p kt f", f=D))

        kT_v = kT_d.ap().rearrange("p (kt s) -> p kt s", s=K_KT * P)
        vT_v = vT_d.ap().rearrange("p (kt s) -> p kt s", s=V_KT * P)

        # ---------- contribution projections + gathers ----------
        with tc.tile_pool(name="ctr", bufs=1) as ctr:
            xkc = ctr.tile([P, kt_in, K_GATH * P], BF16, tag="xkc")
            nc.sync.dma_start(xkc[:], kT_v[:, :, :K_GATH * P])
            xvc = ctr.tile([P, kt_in, V_GATH * P], BF16, tag="xvc")
            nc.sync.dma_start(xvc[:], vT_v[:, :, :V_GATH * P])
            khT_c = ctr.tile([P, 6, K_GATH * P], BF16, tag="khT_c")
            vh_c = ctr.tile([P, V_GATH, 6 * P], BF16, tag="vh_c")

            # K contribution tiles {16+c}, {24+c}
            for ft in range(6):
                ps = psA.tile([P, EXP_BATCH * P], F32, tag="scores")
                for kt in range(kt_in):
                    nc.tensor.matmul(
                        ps[:, :K_GATH * P],
                        wk_sb[:, kt, ft * P:(ft + 1) * P],
                        xkc[:, kt, :],
                        start=(kt == 0), stop=(kt == kt_in - 1))
                nc.scalar.copy(khT_c[:, ft, :], ps[:, :K_GATH * P])
            for b in range(K_GATH):
                nc.sync.dma_start(
                    cc[("k2", "k3")[b], "in"].ap().rearrange(
                        "p (ft w) -> p ft w", w=P),
                    khT_c[:, :, b * P:(b + 1) * P])

            # V contribution tiles {16+c}, {24+c}
            for b in range(V_GATH):
                for half in range(2):
                    ps = psA.tile([P, EXP_BATCH * P], F32, tag="scores")
                    for kt in range(kt_in):
                        nc.tensor.matmul(
                            ps[:, :384],
                            xvc[:, kt, b * P:(b + 1) * P],
                            wv_sb[:, kt, half * 384:(half + 1) * 384],
                            start=(kt == 0), stop=(kt == kt_in - 1))
                    nc.vector.tensor_copy(
                        vh_c[:, b, half * 384:(half + 1) * 384],
                        ps[:, :384])
                nc.sync.dma_start(cc[("v2", "v3")[b], "in"].ap(),
                                  vh_c[:, b, :])

            for name in ("k2", "k3", "v2", "v3"):
                nc.gpsimd.collective_compute(
                    "AllGather", mybir.AluOpType.bypass,
                    replica_groups=[list(range(N_CORES))],
                    ins=[cc[name, "in"].ap()],
                    outs=[cc[name, "out"].ap()],
                )

            # Q projection: weight streamed in two half-width pieces
            xq = ctr.tile([P, kt_in, NQ], BF16, tag="xq", bufs=1)
            nc.sync.dma_start(xq[:], qT_d.ap().rearrange(
                "p (kt s) -> p kt s", s=NQ))
            for piece in range(2):
                wq_sb = ctr.tile([P, kt_in, 384], BF16, tag="wq",
                                 name="wq_sb", bufs=2)
                nc.gpsimd.dma_start(
                    wq_sb[:],
                    wqT_d.ap().rearrange("p (kt f) -> p kt f", f=D)
                    [:, :, piece * 384:(piece + 1) * 384])
                for fh in range(3):
                    ft = piece * 3 + fh
                    ps = psA.tile([P, EXP_BATCH * P], F32, tag="scores")
                    for kt in range(kt_in):
                        nc.tensor.matmul(
                            ps[:, :NQ],
                            wq_sb[:, kt, fh * P:(fh + 1) * P],
                            xq[:, kt, :],
                            start=(kt == 0), stop=(kt == kt_in - 1))
                    nc.vector.tensor_copy(qhT[:, ft, :], ps[:, :NQ])

        nc.vector.memset(vh[:, :, :, DH:], 1.0)

        # gathered bands into SBUF, in collective-arrival order
        for bi, name in enumerate(("k2", "k3")):
            for c in range(N_CORES):
                gkt = 16 + 8 * bi + c
                nc.sync.dma_start(
                    khT[:, :, gkt * P:(gkt + 1) * P],
                    cc[name, "out"][c * P:(c + 1) * P, :].rearrange(
                        "p (ft w) -> p ft w", w=P))
        for bi, name in enumerate(("v2", "v3")):
            for c in range(N_CORES):
                gkt = 16 + 8 * bi + c
                nc.sync.dma_start(
                    vh[:, gkt, :, :DH],
                    cc[name, "out"][c * P:(c + 1) * P, :].rearrange(
                        "p (hh w) -> p hh w", w=DH))

        # replicated x chunks stream in per half-band (4 tiles)
        def xk_chunk(band, hb):
            xk = st.tile([P, kt_in, 4 * P], BF16, tag="xk", name="xk",
                         bufs=2)
            lo = (K_GATH + 8 * band + 4 * hb) * P
            nc.gpsimd.dma_start(xk[:], kT_v[:, :, lo:lo + 4 * P])
            return xk

        def xv_chunk(band, hb):
            xv = st.tile([P, kt_in, 4 * P], BF16, tag="xv", name="xv",
                         bufs=2)
            lo = (V_GATH + 8 * band + 4 * hb) * P
            nc.gpsimd.dma_start(xv[:], vT_v[:, :, lo:lo + 4 * P])
            return xv

        def k_repl(band, ft, xk2):
            # project K band tiles (8) for feature chunk ft into khT
            for hb in range(2):
                ps = psA.tile([P, EXP_BATCH * P], F32, tag="scores")
                for kt in range(kt_in):
                    nc.tensor.matmul(
                        ps[:, :4 * P],
                        wk_sb[:, kt, ft * P:(ft + 1) * P],
                        xk2[hb][:, kt, :],
                        start=(kt == 0), stop=(kt == kt_in - 1))
                nc.vector.tensor_copy(
                    khT[:, ft, (8 * band + 4 * hb) * P:
                        (8 * band + 4 * hb + 4) * P],
                    ps[:, :4 * P])

        def v_repl(band, half, xv2):
            # project V band tiles (8), heads half*6..half*6+5, into vh
            for hb in range(2):
                for t in range(4):
                    ps = psA.tile([P, EXP_BATCH * P], F32, tag="scores")
                    for kt in range(kt_in):
                        nc.tensor.matmul(
                            ps[:, :384],
                            xv2[hb][:, kt, t * P:(t + 1) * P],
                            wv_sb[:, kt, half * 384:(half + 1) * 384],
                            start=(kt == 0), stop=(kt == kt_in - 1))
                    nc.vector.tensor_copy(
                        vh[:, 8 * band + 4 * hb + t,
                           half * 6:(half + 1) * 6, :DH],
                        ps[:, :384].rearrange("p (hh w) -> p hh w", w=DH))

        # ---------- attention ----------
        if has_bias:
            # bias chunk of the out-projection: row 0 = 1 -> adds wo2 row 0
            nc.vector.memset(aT2[:, ho2 - 1, :], 0.0)
            nc.vector.memset(aT2[0:1, ho2 - 1, :], 1.0)

        with tc.tile_pool(name="expp", bufs=2) as expp, \
             tc.tile_pool(name="mskp", bufs=1) as mskp, \
             tc.tile_pool(name="smp", bufs=4) as smp, \
             tc.tile_pool(name="wop", bufs=1) as wop, \
             tc.tile_pool(name="outp", bufs=2) as outp:
            tri = mskp.tile([P, 8 * P], BF16, tag="tri")
            nc.gpsimd.dma_start(tri[:], mask_d.ap())
            ident = mskp.tile([P, P], BF16, tag="ident")
            nc.gpsimd.dma_start(ident[:], ident_d.ap())
            wo_sb = wop.tile([P, ho2, D], BF16, tag="wo")
            nc.gpsimd.dma_start(wo_sb[:], wo2_d.ap().rearrange(
                "p (h f) -> p h f", f=D))

            W = SLOT_NKT[-1] * P            # per-head exp region
            PB = 4                           # kt per pair-batch

            def scores_block(j, hp, expb):
                """scores + exp + causal mask for head pair hp of slot j."""
                nkt = SLOT_NKT[j]
                ft = hp
                done = 0
                while done < nkt:
                    nb = min(PB, nkt - done)
                    ps = psA.tile([P, 2 * PB * P], F32, tag="scores")
                    for b in range(nb):
                        kt = done + b
                        # even head on PE rows 0-63, odd on 64-127:
                        # adjacent issues run concurrently
                        nc.tensor.matmul(
                            ps[:, b * P:(b + 1) * P],
                            khT[0:DH, ft, kt * P:(kt + 1) * P],
                            qhT[0:DH, ft, j * P:(j + 1) * P],
                            start=True, stop=True)
                        nc.tensor.matmul(
                            ps[:, (PB + b) * P:(PB + b + 1) * P],
                            khT[DH:P, ft, kt * P:(kt + 1) * P],
                            qhT[DH:P, ft, j * P:(j + 1) * P],
                            start=True, stop=True)
                    # one strided exp over both heads' sub-batches
                    nc.scalar.activation(
                        expb[:].rearrange("p (g w) -> p g w", g=2)
                        [:, :, done * P:(done + nb) * P],
                        ps[:].rearrange("p (g w) -> p g w", g=2)
                        [:, :, :nb * P],
                        mybir.ActivationFunctionType.Exp,
                        scale=0.125)
                    done += nb
                for base in (0, W):
                    # one wide multiply over the slot's last 8 key-tiles:
                    # the per-core mask data carries ones before the
                    # diagonal, the triangle at it, zeros after it
                    lo = base + (nkt - 8) * P
                    nc.vector.tensor_mul(
                        expb[:, lo:lo + 8 * P],
                        expb[:, lo:lo + 8 * P],
                        tri[:, :])

            def pv_block(j, hp, expb):
                """PV + normalize + transpose into aT2 for head pair hp."""
                nkt = SLOT_NKT[j]
                he, hu = 2 * hp, 2 * hp + 1
                poT = psB.tile([P, 2 * VW], F32, tag="pv")
                for hi, (h, base) in enumerate(((he, 0), (hu, W))):
                    for kt in range(nkt):
                        nc.tensor.matmul(
                            poT[:, hi * VW:(hi + 1) * VW],
                            expb[:, base + kt * P:base + (kt + 1) * P],
                            vh[:, kt, h, :],
                            start=(kt == 0), stop=(kt == nkt - 1))
                # denominators sit at per-q partitions: cols 64 and 129
                rec = smp.tile([P, 2], F32, tag="rec")
                nc.vector.tensor_copy(rec[:, 0:1], poT[:, DH:DH + 1])
                nc.vector.tensor_copy(rec[:, 1:2], poT[:, VW + DH:VW + DH + 1])
                nc.vector.reciprocal_approx_fast(rec[:, :], rec[:, :])
                nrm = smp.tile([P, P], BF16, tag="nrm")
                nc.vector.tensor_scalar_mul(nrm[:, 0:DH], poT[:, 0:DH],
                                            rec[:, 0:1])
                nc.vector.tensor_scalar_mul(nrm[:, DH:P],
                                            poT[:, VW:VW + DH],
                                            rec[:, 1:2])
                tp = psB.tile([P, P], BF16, tag="pv", name="tp")
                nc.tensor.transpose(tp[:, :], nrm[:, :], ident[:, :])
                nc.vector.tensor_copy(aT2[:, hp, j * P:(j + 1) * P],
                                      tp[:, :P])

            def outproj(j):
                osb = outp.tile([P, D], F32, tag="osb")
                for half in range(2):
                    pw2 = psB.tile([P, 384], F32, tag="pv", name="pw2")
                    for p2 in range(ho2):
                        nc.tensor.matmul(
                            pw2[:, :384],
                            aT2[:, p2, j * P:(j + 1) * P],
                            wo_sb[:, p2, half * 384:(half + 1) * 384],
                            start=(p2 == 0), stop=(p2 == ho2 - 1))
                    nc.vector.tensor_copy(osb[:, half * 384:(half + 1) * 384],
                                          pw2[:, :384])
                nc.sync.dma_start(out_d[j * P:(j + 1) * P, :], osb[:])

            # prefetch first K band's x chunks
            xk2 = [xk_chunk(0, 0), xk_chunk(0, 1)]
            xv2 = None
            for j in range(N_SLOTS):
                expbs = {}
                expbs[0] = expp.tile([P, 2 * W], BF16, tag="expb", name="e0")
                if j < K_REPL_BANDS:
                    k_repl(j, 0, xk2)
                scores_block(j, 0, expbs[0])
                for hp in range(H // 2):
                    nxt = hp + 1
                    if nxt < H // 2:
                        # stagger: emit scores (and the K chunk they need)
                        # one head-pair ahead of PV
                        expbs[nxt] = expp.tile([P, 2 * W], BF16,
                                               tag="expb", name="e")
                        if j < K_REPL_BANDS:
                            k_repl(j, nxt, xk2)
                        scores_block(j, nxt, expbs[nxt])
                    # V replication for this slot's band, half at a time,
                    # placed just before the PV that first consumes it
                    if j < V_REPL_BANDS:
                        if hp == 0:
                            xv2 = [xv_chunk(j, 0), xv_chunk(j, 1)]
                            v_repl(j, 0, xv2)
                        elif hp == 3:
                            v_repl(j, 1, xv2)
                    pv_block(j, hp, expbs.pop(hp))
                if j + 1 < K_REPL_BANDS:
                    xk2 = [xk_chunk(j + 1, 0), xk_chunk(j + 1, 1)]
                outproj(j)


# ------------------------------------------------------------------
# host side
# ------------------------------------------------------------------

_CACHE = {}


def _get_nc(has_bias):
    key = has_bias
    if key not in _CACHE:
        _CACHE[key] = build(has_bias)
    return _CACHE[key]


def _bf16(x):
    return np.asarray(x, dtype=ml_dtypes.bfloat16)


def _build_mask(core):
    # Applied to the last 8 key-tiles of every slot (tiles 8j..8j+7, the
    # diagonal band): ones before this core's diagonal tile, transposed
    # lower-triangle at it (keep key_row <= q_col), zeros beyond.
    m = np.zeros((P, 8 * P), dtype=np.float32)
    m[:, :core * P] = 1.0
    m[:, core * P:(core + 1) * P] = np.tril(
        np.ones((P, P), dtype=np.float32)).T
    return _bf16(m)


def prepare_in_maps(q, k, v, wq, bq, wk, bk, wv, bv, wo, bo, has_bias):
    kt_in = 7 if has_bias else 6
    ho2 = 7 if has_bias else 6
    d_in = kt_in * P

    def _fold(x2d):
        # [kt_in*128, n] -> [128, kt_in*n] partition-major contiguous
        n = x2d.shape[1]
        return np.ascontiguousarray(
            x2d.reshape(kt_in, P, n).transpose(1, 0, 2).reshape(P, kt_in * n))

    def aug(xT, bias_row):
        # [768, n] -> folded [128, kt_in*n] with ones row at 768 (inputs)
        if not has_bias:
            return _bf16(_fold(xT))
        out = np.zeros((d_in, xT.shape[1]), dtype=np.float32)
        out[:D] = xT
        out[D] = bias_row
        return _bf16(_fold(out))

    def augw(w, b):
        # torch Linear weight [out, in] -> folded lhsT with bias row
        wT = w.T.astype(np.float32)
        if not has_bias:
            return _bf16(_fold(wT))
        out = np.zeros((d_in, D), dtype=np.float32)
        out[:D] = wT
        out[D] = b
        return _bf16(_fold(out))

    wqT = augw(wq, bq); wkT = augw(wk, bk); wvT = augw(wv, bv)

    # out-proj weight: chunk p rows 64*par+d = wo[:, (2p+par)*64+d]
    woT = wo.T.astype(np.float32)                       # [in, out]
    wo2 = np.zeros((P, ho2, D), dtype=np.float32)
    wo2[:, :6, :] = woT.reshape(6, 2, DH, D).transpose(1, 2, 0, 3).reshape(
        P, 6, D)
    if has_bias:
        wo2[0, 6, :] = bo
    wo2 = _bf16(np.ascontiguousarray(wo2.reshape(P, ho2 * D)))

    ident = _bf16(np.eye(P, dtype=np.float32))

    q2 = q[0].astype(np.float32)   # [S, D]
    k2 = k[0].astype(np.float32)
    v2 = v[0].astype(np.float32)

    in_maps = []
    for c in range(N_CORES):
        rows = np.concatenate(
            [np.arange(t * P, (t + 1) * P) for t in q_tiles_of_core(c)])
        k_tiles = [16 + c, 24 + c] + list(range(8 * K_REPL_BANDS))
        v_tiles = [16 + c, 24 + c] + list(range(8 * V_REPL_BANDS))
        krows = np.concatenate(
            [np.arange(t * P, (t + 1) * P) for t in k_tiles])
        vrows = np.concatenate(
            [np.arange(t * P, (t + 1) * P) for t in v_tiles])
        qT = aug(q2[rows].T, 1.0)
        kT = aug(k2[krows].T, 1.0)
        vT = aug(v2[vrows].T, 1.0)
        in_maps.append({
            "qT": qT, "kT": kT, "vT": vT,
            "wqT": wqT, "wkT": wkT, "wvT": wvT, "wo2": wo2,
            "mask": _build_mask(c), "ident": ident,
        })
    return in_maps


def kernel(q, k, v, wq, bq, wk, bk, wv, bv, wo, bo):
    q = np.asarray(q); k = np.asarray(k); v = np.asarray(v)
    wq = np.asarray(wq); wk = np.asarray(wk); wv = np.asarray(wv)
    wo = np.asarray(wo)
    bq = np.asarray(bq); bk = np.asarray(bk); bv = np.asarray(bv)
    bo = np.asarray(bo)
    has_bias = any(np.any(b) for b in (bq, bk, bv, bo))
    nc = _get_nc(has_bias)
    in_maps = prepare_in_maps(q, k, v, wq, bq, wk, bk, wv, bv, wo, bo,
                              has_bias)

    res = bass_utils.run_bass_kernel_spmd(
        nc, in_maps, core_ids=list(range(N_CORES)))
    kernel.last_exec_time_ns = res.exec_time_ns

    out = np.empty((S, D), dtype=np.float32)
    for c in range(N_CORES):
        for j, t in enumerate(q_tiles_of_core(c)):
            out[t * P:(t + 1) * P] = res.results[c]["out"][j * P:(j + 1) * P]
    return out.reshape(1, S, D)
